# revision 1
# baseline (speedup 1.0000x reference)
"""CRF negative-log-likelihood loss kernel for Trainium2 (Bass/Tile).

Problem: B=4096 sequences, L=4096 positions, T=2 tags, mask all-ones.
Reference: mean over batch of (logZ - gold_score) / L.

Strategy (pure data parallel, 8 cores, 512 sequences each):
  * Normalizer logZ via the transfer-matrix product
        P = M_{L-1} @ ... @ M_1 @ diag(exp(start + e_0)),
    M_l = diag(exp(e_l)) @ exp(Tr), computed as a balanced tree reduction
    in the *linear* (probability) domain up to 64-position blocks.  Every
    exp() carries a -1.0 bias so block entries stay ~exp(N(0, sigma)); the
    exact bias correction (+1.0 per position) is added back on the host.
    Blocks are converted to log domain (Ln) and combined with
    log-semiring 2x2 products (max + softplus) up the rest of the tree.
  * Level-0 pair combine uses the rank-1 structure
        (M_odd @ M_even)[i,j] = X_i * G1_ij * (r_ij * a + b)
    which is 2 fused scalar_tensor_tensor ops per matrix entry.
  * Gold score in closed form (T=2 => Tr[a,b] bilinear in a,b):
        gold = sum(e0) + sum(t*(e1-e0)) + cC*sum(t_l*t_{l-1})
             + cAB*sum(t) + ct0*t_0 + ctL*t_{L-1} + const
    computed with fused accumulating ops.
  * Work is split DVE / GPSIMD / ACT so no single engine is the wall.

The kernel is self-contained: shapes/sharding are hardcoded for the
4096x4096x2 problem; tags are narrowed to int32 host-side (values in
{0,1}); the mask is validated to be all-ones (guaranteed by the problem
spec) with a numpy fallback otherwise.
"""

import math
from contextlib import ExitStack

import numpy as np

import concourse.bass as bass
import concourse.tile as tile
from concourse import mybir
from concourse.bass_utils import run_bass_kernel_spmd

AF = mybir.ActivationFunctionType
OP = mybir.AluOpType
F32 = mybir.dt.float32
I32 = mybir.dt.int32
I8 = mybir.dt.int8

N_CORES = 8
P = 128          # SBUF partitions


def _ap(t, off, dims):
    """Custom AP on SBUF tile t: partition dim + given [step, count] dims."""
    base = t[:]
    return bass.AP(tensor=base.tensor, offset=base.offset + off,
                   ap=[base.ap[0]] + [list(d) for d in dims])


def _split_multiwaits(nc):
    """This container's walrus accepts only ONE sem wait per instruction;
    Tile's tail drain carries several.  Hoist extra waits onto same-engine
    single-wait drains inserted immediately before the instruction."""
    for f in nc.m.functions:
        for b in f.blocks:
            out = []
            changed = False
            for ins in b.instructions:
                si = ins.sync_info
                if si is not None and si.on_wait and len(si.on_wait) > 1:
                    waits = list(si.on_wait)
                    for k, w in enumerate(waits[:-1]):
                        d = mybir.InstDrain(name=f"{ins.name}-wsplit{k}")
                        d.engine = ins.engine
                        d.sync_info = mybir.SyncInfo(on_wait=[w], on_update=[])
                        nc.register_instruction(d, overwrite=True)
                        out.append(d)
                    ins.sync_info = mybir.SyncInfo(
                        on_wait=[waits[-1]], on_update=list(si.on_update or []))
                    changed = True
                out.append(ins)
            if changed:
                b.instructions = out
    return nc


def _build(consts, G, L, C, BLK, debug=False, ablate=(), repeat=1):
    """Build the Bass program for one core: G groups of 128 sequences."""
    (K4, E4, goldc, cAB, cC, ct0, ctL, r_e, g1_e, CBIAS) = consts
    NCH = L // C          # chunks per group
    K1 = C // 2           # level-0 output matrices per chunk
    NLEV = int(math.log2(BLK)) - 1   # generic linear levels (1..NLEV)
    BPC = C // BLK        # blocks per chunk
    NBLK = L // BLK       # blocks per group
    ULEV = int(math.log2(NBLK))      # upper (log-domain) levels
    NCONST = 16

    nc = bass.Bass()
    em = nc.dram_tensor("emissions", [G * P, L, 2], F32, kind="ExternalInput")
    tg = nc.dram_tensor("tags", [G * P, L], I8, kind="ExternalInput")
    cst = nc.dram_tensor("consts", [1, NCONST], F32, kind="ExternalInput")
    nll = nc.dram_tensor("nll", [G, P], F32, kind="ExternalOutput")
    if debug:
        zdbg = nc.dram_tensor("zdbg", [G, P], F32, kind="ExternalOutput")
        gdbg = nc.dram_tensor("gdbg", [G, P], F32, kind="ExternalOutput")

    with tile.TileContext(nc) as tc, ExitStack() as ctx:
        io = ctx.enter_context(tc.tile_pool(name="io", bufs=2))
        wk = ctx.enter_context(tc.tile_pool(name="wk", bufs=2))
        l0p = ctx.enter_context(tc.tile_pool(name="l0p", bufs=3))
        ps = ctx.enter_context(tc.tile_pool(name="ps", bufs=1))

        # Persistent tiles
        CST = ps.tile([P, NCONST], F32, tag="cst")
        nc.sync.dma_start(out=CST, in_=bass.AP(
            tensor=cst[:].tensor, offset=0, ap=[[0, P], [1, NCONST]]))
        LOG = ps.tile([P, G * 4 * NBLK], F32, tag="log")       # block logs
        ACC = ps.tile([P, G * 4 * NCH], F32, tag="acc")        # gold accums
        TEF = ps.tile([P, 2 * G], F32, tag="tef")              # t0 / tLast
        BIASN = ps.tile([P, 1], F32, tag="biasn")              # -CBIAS for Exp
        nc.vector.memset(BIASN, -CBIAS)

        for _rep in range(repeat):
            for g in range(G):
                for c in range(NCH):
                    l0 = c * C
                    # ---- loads ----
                    E = io.tile([P, 2 * C], F32, tag="E")
                    nc.sync.dma_start(out=_ap(E, 0, [[2, C], [1, 2]]),
                                      in_=em[g * P:(g + 1) * P, l0:l0 + C, :])
                    ov = 0 if c == 0 else 1        # overlap 1 tag col for l-1
                    TG = io.tile([P, C + 1], I8, tag="TG")
                    nc.sync.dma_start(out=TG[:, :C + ov],
                                      in_=tg[g * P:(g + 1) * P, l0 - ov:l0 + C])

                    # ---- exp(e - CBIAS)  (ACT) ----
                    EX = l0p.tile([P, 2 * C], F32, tag="EX")
                    nc.scalar.activation(EX, E, AF.Exp, bias=BIASN[:, 0:1], scale=1.0)

                    # ---- gold-score pieces ----
                    if "gold" in ablate:
                        pass

                    if "gold" not in ablate:
                        D = wk.tile([P, C], F32, tag="D")         # e1 - e0
                        nc.gpsimd.tensor_tensor(out=D, in0=_ap(E, 1, [[2, C]]),
                                                in1=_ap(E, 0, [[2, C]]), op=OP.subtract)
                        SCA = wk.tile([P, C], F32, tag="SCX")
                        # sum e0 (ACT, fused accumulate)
                        nc.scalar.activation(SCA, _ap(E, 0, [[2, C]]), AF.Copy,
                                             accum_out=ACC[:, (g * 4 + 0) * NCH + c:(g * 4 + 0) * NCH + c + 1])
                        # tags cast to f32 (Pool can't read int32)
                        TF = wk.tile([P, C + 1], F32, tag="TF")
                        nc.scalar.activation(TF[:, :C + ov], TG[:, :C + ov], AF.Copy)
                        SCB = wk.tile([P, C], F32, tag="SCX")
                        # sum t (ACT copy, fused accumulate)
                        nc.scalar.activation(SCB, TF[:, ov:ov + C], AF.Copy,
                                             accum_out=ACC[:, (g * 4 + 2) * NCH + c:(g * 4 + 2) * NCH + c + 1])
                        SCC = wk.tile([P, C], F32, tag="SCP")
                        # sum t*(e1-e0): POOL product, ACT accumulating copy
                        nc.gpsimd.tensor_tensor(out=SCC, in0=TF[:, ov:ov + C], in1=D,
                                                op=OP.mult)
                        SCC2 = wk.tile([P, C], F32, tag="SCX")
                        nc.scalar.activation(SCC2, SCC, AF.Copy,
                                             accum_out=ACC[:, (g * 4 + 1) * NCH + c:(g * 4 + 1) * NCH + c + 1])
                        SCD = wk.tile([P, C], F32, tag="SCP")
                        # sum t_l * t_{l-1} (covers chunk seam via overlap col)
                        npair = C - 1 + ov
                        nc.gpsimd.tensor_tensor(out=SCD[:, :npair], in0=TF[:, 1:1 + npair],
                                                in1=TF[:, :npair], op=OP.mult)
                        SCD2 = wk.tile([P, C], F32, tag="SCX")
                        nc.scalar.activation(SCD2[:, :npair], SCD[:, :npair], AF.Copy,
                                             accum_out=ACC[:, (g * 4 + 3) * NCH + c:(g * 4 + 3) * NCH + c + 1])
                        if c == 0:   # first tag
                            nc.scalar.activation(TEF[:, g:g + 1], TF[:, 0:1], AF.Copy)
                        if c == NCH - 1:  # last tag
                            nc.scalar.activation(TEF[:, G + g:G + g + 1], TF[:, C + ov - 1:C + ov], AF.Copy)

                    # ---- level 0: pair combine via rank-1 structure ----
                    if "tree" in ablate:
                        continue
                    U = l0p.tile([P, 4 * K1], F32, tag="U")
                    C0 = l0p.tile([P, 4 * K1], F32, tag="C0")
                    for e in range(4):
                        i = e // 2
                        # u_e = r_e * a + b
                        nc.vector.scalar_tensor_tensor(
                            out=U[:, e * K1:(e + 1) * K1],
                            in0=_ap(EX, 0, [[4, K1]]), scalar=float(r_e[e]),
                            in1=_ap(EX, 1, [[4, K1]]), op0=OP.mult, op1=OP.add)
                        # C0_e = (u_e * g1_e) * X_i   (X_0 = c-hat, X_1 = d-hat)
                        nc.vector.scalar_tensor_tensor(
                            out=C0[:, e * K1:(e + 1) * K1],
                            in0=U[:, e * K1:(e + 1) * K1], scalar=float(g1_e[e]),
                            in1=_ap(EX, 2 + i, [[4, K1]]), op0=OP.mult, op1=OP.mult)
                    if c == 0:
                        # patch k=0: C0[:,e*K1] = (exp(a0) column scale) form:
                        # R[i,j] = ehat1[i] * That[i,j]*shat[j] * ehat0[j]
                        P4 = wk.tile([P, 4], F32, tag="P4")
                        nc.vector.tensor_tensor(
                            out=P4, in0=_ap(EX, 2, [[1, 2], [0, 2]]),
                            in1=_ap(EX, 0, [[0, 2], [1, 2]]), op=OP.mult)
                        nc.vector.tensor_tensor(
                            out=_ap(C0, 0, [[K1, 4]]), in0=P4,
                            in1=CST[:, 0:4], op=OP.mult)

                    # ---- generic linear levels (GPSIMD for level 1, DVE rest) ----
                    prev, kp = C0, K1
                    for v in range(1, NLEV + 1):
                        k = kp // 2
                        cur = wk.tile([P, 4 * k], F32, tag=f"L{v}")
                        tmp = wk.tile([P, 4 * k], F32, tag=f"T{v}")
                        eng = nc.gpsimd if v >= 4 else nc.vector
                        out_ap = _ap(cur, 0, [[2 * k, 2], [k, 2], [1, k]])
                        tmp_ap = _ap(tmp, 0, [[2 * k, 2], [k, 2], [1, k]])
                        # A[i,mu] at e=2i+mu (odd m), B[mu,j] at e=2mu+j (even m)
                        a0 = _ap(prev, 0 * kp + 1, [[2 * kp, 2], [0, 2], [2, k]])
                        b0 = _ap(prev, 0 * kp + 0, [[0, 2], [kp, 2], [2, k]])
                        a1 = _ap(prev, 1 * kp + 1, [[2 * kp, 2], [0, 2], [2, k]])
                        b1 = _ap(prev, 2 * kp + 0, [[0, 2], [kp, 2], [2, k]])
                        eng.tensor_tensor(out=out_ap, in0=a0, in1=b0, op=OP.mult)
                        eng.tensor_tensor(out=tmp_ap, in0=a1, in1=b1, op=OP.mult)
                        eng.tensor_tensor(out=cur, in0=cur, in1=tmp, op=OP.add)
                        prev, kp = cur, k

                    # ---- block log conversion ----
                    nc.scalar.activation(
                        _ap(LOG, g * 4 * NBLK + c * BPC, [[NBLK, 4], [1, BPC]]),
                        prev, AF.Ln)

            # ---- upper tree in log domain, all groups batched per level ----
            if "upper" not in ablate:
                GN = G * 4 * NBLK
                prev_t, prev_gs, kp = LOG, 4 * NBLK, NBLK
                for v in range(ULEV):
                    k = kp // 2
                    s0 = wk.tile([P, G * 4 * k], F32, tag=f"US0{v}")
                    s1 = wk.tile([P, G * 4 * k], F32, tag=f"US1{v}")
                    cur = wk.tile([P, G * 4 * k], F32, tag=f"UC{v}")
                    mx = wk.tile([P, G * 4 * k], F32, tag=f"UM{v}")
                    mn = wk.tile([P, G * 4 * k], F32, tag=f"UN{v}")

                    def _oap(t):
                        return _ap(t, 0, [[4 * k, G], [2 * k, 2], [k, 2], [1, k]])

                    def _a(mu):
                        return _ap(prev_t, mu * kp + 1,
                                   [[prev_gs, G], [2 * kp, 2], [0, 2], [2, k]])

                    def _b(mu):
                        return _ap(prev_t, 2 * mu * kp,
                                   [[prev_gs, G], [0, 2], [kp, 2], [2, k]])

                    nc.vector.tensor_tensor(out=_oap(s0), in0=_a(0), in1=_b(0), op=OP.add)
                    nc.vector.tensor_tensor(out=_oap(s1), in0=_a(1), in1=_b(1), op=OP.add)
                    nc.vector.tensor_tensor(out=mx, in0=s0, in1=s1, op=OP.max)
                    nc.vector.tensor_tensor(out=mn, in0=s0, in1=s1, op=OP.min)
                    nc.vector.tensor_tensor(out=mn, in0=mn, in1=mx, op=OP.subtract)
                    nc.scalar.activation(mn, mn, AF.Exp)
                    nc.scalar.activation(mn, mn, AF.Ln, bias=1.0)  # ln(1+exp(d))
                    nc.vector.tensor_tensor(out=cur, in0=mx, in1=mn, op=OP.add)
                    prev_t, prev_gs, kp = cur, 4 * k, k

                # ---- finalize logZ for all groups: lse over 4 entries + end ----
                ZT = wk.tile([P, G * 4], F32, tag="ZT")
                nc.vector.tensor_tensor(
                    out=ZT, in0=prev_t,
                    in1=bass.AP(tensor=CST[:].tensor, offset=CST[:].offset + 4,
                                ap=[CST[:].ap[0], [0, G], [1, 4]]),
                    op=OP.add)
                ZM = wk.tile([P, G], F32, tag="ZM")
                nc.vector.tensor_reduce(out=ZM, in_=_ap(ZT, 0, [[4, G], [1, 4]]),
                                        axis=mybir.AxisListType.X, op=OP.max)
                ZS = wk.tile([P, G * 4], F32, tag="ZS")
                nc.vector.tensor_tensor(out=ZS, in0=ZT,
                                        in1=_ap(ZM, 0, [[1, G], [0, 4]]), op=OP.subtract)
                nc.scalar.activation(ZS, ZS, AF.Exp)
                ZP = wk.tile([P, G], F32, tag="ZP")
                nc.vector.tensor_reduce(out=ZP, in_=_ap(ZS, 0, [[4, G], [1, 4]]),
                                        axis=mybir.AxisListType.X, op=OP.add)
                nc.scalar.activation(ZP, ZP, AF.Ln)
                Z = wk.tile([P, G], F32, tag="Z")
                nc.vector.tensor_tensor(out=Z, in0=ZP, in1=ZM, op=OP.add)

                # ---- gold score, all groups ----
                SM = wk.tile([P, G * 4], F32, tag="SM")   # [g][type]
                nc.vector.tensor_reduce(
                    out=SM, in_=_ap(ACC, 0, [[4 * NCH, G], [NCH, 4], [1, NCH]]),
                    axis=mybir.AxisListType.X, op=OP.add)
                G1 = wk.tile([P, G], F32, tag="G1")
                G2 = wk.tile([P, G], F32, tag="G2")
                # cAB*sum(t) + sum(e0)
                nc.vector.scalar_tensor_tensor(out=G1, in0=_ap(SM, 2, [[4, G]]),
                                               scalar=cAB, in1=_ap(SM, 0, [[4, G]]),
                                               op0=OP.mult, op1=OP.add)
                # cC*sum(tt) + sum(t*d)
                nc.vector.scalar_tensor_tensor(out=G2, in0=_ap(SM, 3, [[4, G]]),
                                               scalar=cC, in1=_ap(SM, 1, [[4, G]]),
                                               op0=OP.mult, op1=OP.add)
                nc.vector.tensor_tensor(out=G1, in0=G1, in1=G2, op=OP.add)
                nc.vector.scalar_tensor_tensor(out=G1, in0=TEF[:, 0:G], scalar=ct0,
                                               in1=G1, op0=OP.mult, op1=OP.add)
                nc.vector.scalar_tensor_tensor(out=G1, in0=TEF[:, G:2 * G], scalar=ctL,
                                               in1=G1, op0=OP.mult, op1=OP.add)
                nc.vector.tensor_scalar(out=G1, in0=G1, scalar1=goldc, scalar2=None,
                                        op0=OP.add)
                # nll = (Z - gold) / L
                NL = wk.tile([P, G], F32, tag="NL")
                nc.vector.tensor_tensor(out=NL, in0=Z, in1=G1, op=OP.subtract)
                nc.vector.tensor_scalar(out=NL, in0=NL, scalar1=1.0 / L, scalar2=None,
                                        op0=OP.mult)
                nc.sync.dma_start(
                    out=bass.AP(tensor=nll[:].tensor, offset=0, ap=[[1, P], [P, G]]),
                    in_=NL)
                if debug:
                    nc.sync.dma_start(
                        out=bass.AP(tensor=zdbg[:].tensor, offset=0, ap=[[1, P], [P, G]]),
                        in_=Z)
                    nc.sync.dma_start(
                        out=bass.AP(tensor=gdbg[:].tensor, offset=0, ap=[[1, P], [P, G]]),
                        in_=G1)



    return _split_multiwaits(nc)


_CACHE = {}
LAST_RESULTS = None


def _get_nc(key, consts, G, L, C, BLK):
    if key not in _CACHE:
        _CACHE[key] = _build(consts, G, L, C, BLK)
    return _CACHE[key]


def _host_consts(transitions, start_transitions, end_transitions, L, CBIAS=1.0):
    tr = np.asarray(transitions, np.float64)
    st = np.asarray(start_transitions, np.float64)
    en = np.asarray(end_transitions, np.float64)
    Th = np.exp(tr)
    sh = np.exp(st)
    K4 = np.array([Th[i, j] * sh[j] for i in (0, 1) for j in (0, 1)], np.float64)
    E4 = np.array([en[0], en[0], en[1], en[1]], np.float64)
    A = tr[1, 0] - tr[0, 0]
    Bc = tr[0, 1] - tr[0, 0]
    cC = tr[1, 1] - tr[1, 0] - tr[0, 1] + tr[0, 0]
    goldc = (L - 1) * tr[0, 0] + st[0] + en[0]
    cAB = A + Bc
    ct0 = st[1] - st[0] - A
    ctL = en[1] - en[0] - Bc
    G0 = np.array([Th[i, 0] * Th[0, j] for i in (0, 1) for j in (0, 1)])
    G1 = np.array([Th[i, 1] * Th[1, j] for i in (0, 1) for j in (0, 1)])
    r_e = G0 / G1
    return (tuple(np.float32(K4)), tuple(np.float32(E4)), float(np.float32(goldc)),
            float(np.float32(cAB)), float(np.float32(cC)), float(np.float32(ct0)),
            float(np.float32(ctL)), tuple(np.float32(r_e)), tuple(np.float32(G1)),
            float(CBIAS))


def _np_crf_fallback(emissions, tags, mask, transitions, start_transitions,
                     end_transitions):
    """Plain numpy CRF NLL (general mask) — correctness fallback only."""
    em = np.asarray(emissions, np.float64)
    tg = np.asarray(tags, np.int64)
    mk = np.asarray(mask, bool)
    tr = np.asarray(transitions, np.float64)
    st = np.asarray(start_transitions, np.float64)
    en = np.asarray(end_transitions, np.float64)
    B, L, T = em.shape
    score = st[tg[:, 0]] + em[np.arange(B), 0, tg[:, 0]]
    for l in range(1, L):
        emit = em[np.arange(B), l, tg[:, l]]
        trans = tr[tg[:, l], tg[:, l - 1]]
        score += (emit + trans) * mk[:, l]
    alpha = st[None, :] + em[:, 0]
    for l in range(1, L):
        sc = alpha[:, None, :] + tr[None, :, :]
        m = sc.max(axis=2, keepdims=True)
        a_new = np.log(np.exp(sc - m).sum(axis=2)) + m[:, :, 0] + em[:, l]
        alpha = np.where(mk[:, l, None], a_new, alpha)
    m = (alpha + en).max(axis=1, keepdims=True)
    logz = np.log(np.exp(alpha + en - m).sum(axis=1)) + m[:, 0]
    sl = np.maximum(mk.sum(axis=1), 1.0)
    return np.float32(((logz - score) / sl).mean())


def kernel(emissions, tags, mask, transitions, start_transitions,
           end_transitions):
    B, L, T = emissions.shape
    assert T == 2
    if not np.all(mask):
        return _np_crf_fallback(emissions, tags, mask, transitions,
                                start_transitions, end_transitions)

    BS = B // N_CORES
    G = BS // P
    C = 1024
    BLK = 64
    NBLK = L // BLK
    ok_shape = (B % (N_CORES * P) == 0 and L % C == 0 and C % BLK == 0
                and NBLK & (NBLK - 1) == 0 and (C // 2) % 32 == 0)
    if not ok_shape:
        return _np_crf_fallback(emissions, tags, mask, transitions,
                                start_transitions, end_transitions)
    CBIAS = 1.0
    consts = _host_consts(transitions, start_transitions, end_transitions, L,
                          CBIAS)
    key = (consts, G, L, C, BLK)
    nc = _get_nc(key, consts, G, L, C, BLK)

    em = np.ascontiguousarray(emissions, dtype=np.float32)
    tg = np.ascontiguousarray(tags, dtype=np.int8)
    NCONST = 16
    cvec = np.zeros((1, NCONST), np.float32)
    (K4, E4, goldc, cAB, cC, ct0, ctL, r_e, g1_e, _) = consts
    cvec[0, 0:4] = K4
    cvec[0, 4:8] = E4

    in_maps = []
    for c in range(N_CORES):
        in_maps.append({
            "emissions": em[c * BS:(c + 1) * BS],
            "tags": tg[c * BS:(c + 1) * BS],
            "consts": cvec,
        })
    global LAST_RESULTS
    res = run_bass_kernel_spmd(nc, in_maps, core_ids=list(range(N_CORES)))
    LAST_RESULTS = res
    nlls = np.concatenate([r["nll"].reshape(-1) for r in res.results])
    return np.float32(np.mean(nlls, dtype=np.float64) + CBIAS)



# revision 3
# speedup vs baseline: 1.7415x; 1.7415x over previous
"""CRF NLL loss kernel v2 for Trainium2 (Bass/Tile).

B=4096, L=4096, T=2, mask all-ones.  8 cores, data-parallel over batch.

Algorithm (per core, 512 seqs = 4 groups of 128 partitions):
  * Host: fold start_transitions into position-0 emissions; bf16 planar
    layout with pairs bit-reversed within 128-position blocks:
    planes (Ehi0, Ehi1, Elo0, Elo1) each [P, 2048]; tag planes (thi, tlo)
    bf16; tag-only gold terms computed on host.
  * Device per group:
      w-sums  W[(i,b)] = Ehi_i + Elo_b            (1 bf16 TT, 2x mode)
      gold: D = e1-e0 (TT), tD = t*D (pool STT, in place over tags),
            4x-mode TS accumulates
      6 exp streams with transition consts folded into ACT bias
      pair matrices P_ij = G0_ij w_i0 + G1_ij w_i1 (1 TS + 2 TT)
      pair-0 patched to alpha-init leaf (3 tiny ops)
      linear tree: 6 levels of constant-free 2x2 products (3 TT each)
      Ln -> f32 block logs into a persistent per-group buffer
  * Post-loop: log-domain top tree batched across groups (pool + ACT),
    final assembly, one DMA out.
  * Device outputs per seq: lT00, lT10 (log alpha), sum_e0, sum_tD.
    Host: logZ = CB*L + lse(en + lT); gold = sums + host tag part; mean.
"""

import numpy as np
import ml_dtypes
from contextlib import ExitStack

import concourse.bass as bass
import concourse.tile as tile
from concourse import mybir
from concourse.bass_utils import run_bass_kernel_spmd

AF = mybir.ActivationFunctionType
OP = mybir.AluOpType
F32 = mybir.dt.float32
BF16 = mybir.dt.bfloat16
NPBF = ml_dtypes.bfloat16

N_CORES = 8
P = 128            # SBUF partitions
G = 4              # groups of 128 seqs per core
L = 4096
NBLK = 32          # 128-position blocks per sequence
BLKP = 64          # pairs per block
PS = L // 2        # pairs per group-row = plane size (2048)


def _ap(t, off, dims):
    base = t[:]
    return bass.AP(tensor=base.tensor, offset=base.offset + off,
                   ap=[base.ap[0]] + [list(d) for d in dims])


def _split_multiwaits(nc):
    """Walrus here accepts only one sem wait per instruction; hoist extras
    onto same-engine single-wait drains."""
    for f in nc.m.functions:
        for b in f.blocks:
            out = []
            changed = False
            for ins in b.instructions:
                si = ins.sync_info
                if si is not None and si.on_wait and len(si.on_wait) > 1:
                    waits = list(si.on_wait)
                    for k, w in enumerate(waits[:-1]):
                        d = mybir.InstDrain(name=f"{ins.name}-wsplit{k}")
                        d.engine = ins.engine
                        d.sync_info = mybir.SyncInfo(on_wait=[w], on_update=[])
                        nc.register_instruction(d, overwrite=True)
                        out.append(d)
                    ins.sync_info = mybir.SyncInfo(
                        on_wait=[waits[-1]], on_update=list(si.on_update or []))
                    changed = True
                out.append(ins)
            if changed:
                b.instructions = out
    return nc


def _host_consts(transitions, CB):
    tr = np.asarray(transitions, np.float64)
    c = {}
    c["bh"] = tuple(float(tr[i, 0] + tr[0, 0] - 2 * CB) for i in (0, 1))
    c["bc"] = tuple(float(tr[i, 1] + tr[1, 0] - 2 * CB) for i in (0, 1))
    c["bd"] = tuple(float(tr[i, 1] + tr[1, 1] - 2 * CB) for i in (0, 1))
    c["delta"] = float(np.exp(tr[0, 1] - tr[0, 0]))
    c["p1"] = float(np.exp(tr[1, 0] - tr[0, 0]))   # K10/K00
    c["p2"] = float(np.exp(-tr[1, 0]))             # 1/K10
    c["CB"] = float(CB)
    return tuple(sorted(c.items()))


def _build(consts, repeat=1, ablate=()):
    c = dict(consts)
    nc = bass.Bass()
    em = nc.dram_tensor("emissions", [G * P, 4 * PS], BF16, kind="ExternalInput")
    tg = nc.dram_tensor("tagsf", [G * P, 2 * PS], BF16, kind="ExternalInput")
    outp = nc.dram_tensor("outp", [G * P, 4], F32, kind="ExternalOutput")

    with tile.TileContext(nc) as tc, ExitStack() as ctx:
        io = ctx.enter_context(tc.tile_pool(name="io", bufs=2))
        fr = ctx.enter_context(tc.tile_pool(name="fr", bufs=2))
        wk = ctx.enter_context(tc.tile_pool(name="wk", bufs=1))
        ps = ctx.enter_context(tc.tile_pool(name="ps", bufs=1))

        BIAS = ps.tile([P, 8], F32, tag="BIAS")
        bvals = [c["bh"][0], c["bh"][1], c["bc"][0], c["bc"][1],
                 c["bd"][0], c["bd"][1]]
        for k, bv in enumerate(bvals):
            nc.vector.memset(BIAS[:, k:k + 1], float(bv))
        # persistent: block logs for all groups, output accumulators
        LBA = ps.tile([P, 4 * G * NBLK], F32, tag="LBA")   # (i,j)-plane, g, blk
        ACA = ps.tile([P, 4 * G], F32, tag="ACA")

        for _rep in range(repeat):
            for g in range(G):
                EM = io.tile([P, 4 * PS], BF16, tag="EM")
                nc.sync.dma_start(out=EM, in_=em[g * P:(g + 1) * P, :])
                TG = io.tile([P, 2 * PS], BF16, tag="TG")
                nc.sync.dma_start(out=TG, in_=tg[g * P:(g + 1) * P, :])

                # ---- w sums: W[(i,b)] = Ehi_i + Elo_b ----
                W = fr.tile([P, 4 * PS], BF16, tag="W")
                nc.vector.tensor_tensor(
                    out=_ap(W, 0, [[2 * PS, 2], [PS, 2], [1, PS]]),
                    in0=_ap(EM, 0, [[PS, 2], [0, 2], [1, PS]]),
                    in1=_ap(EM, 2 * PS, [[0, 2], [PS, 2], [1, PS]]),
                    op=OP.add)

                # ---- gold (early: fills DVE while ACT does exps) ----
                # PM is declared here; accumulate-op scratch outputs dump
                # into regions that later get fully overwritten (PM) or are
                # dead (D), so no extra tiles are needed.
                PM = wk.tile([P, 4 * PS], BF16, tag="PM")
                D = wk.tile([P, 2 * PS], BF16, tag="D")
                if "gold" in ablate:
                    nc.vector.memset(D, 0.5)
                if "gold" not in ablate:
                 nc.gpsimd.tensor_tensor(
                    out=D,
                    in0=_ap(EM, PS, [[2 * PS, 2], [1, PS]]),
                    in1=_ap(EM, 0, [[2 * PS, 2], [1, PS]]), op=OP.subtract)
                if "gold" not in ablate:
                    nc.scalar.activation(
                        _ap(PM, 0, [[PS, 2], [1, PS]]),
                        _ap(EM, 0, [[2 * PS, 2], [1, PS]]), AF.Copy,
                        accum_out=ACA[:, 4 * g + 2:4 * g + 3])
                    # tD in place over TG (pool)
                    nc.gpsimd.tensor_tensor(out=TG, in0=TG, in1=D, op=OP.mult)
                    nc.scalar.activation(
                        D, TG, AF.Copy,
                        accum_out=ACA[:, 4 * g + 3:4 * g + 4])

                # ---- exp streams: wh0,wh1,wc0,wc1,wd0,wd1 ----
                EX = fr.tile([P, 6 * PS], BF16, tag="EX")
                if "exp" in ablate:
                    nc.scalar.activation(EX[:, 0:PS], W[:, 0:PS], AF.Exp,
                                         bias=BIAS[:, 0:1])
                exp_rng = () if "exp" in ablate else (0, 1)
                for i in exp_rng:
                    nc.scalar.activation(EX[:, i * PS:(i + 1) * PS],
                                         W[:, 2 * i * PS:(2 * i + 1) * PS],
                                         AF.Exp, bias=BIAS[:, i:i + 1])
                for i in exp_rng:
                    nc.scalar.activation(EX[:, (2 + i) * PS:(3 + i) * PS],
                                         W[:, (2 * i + 1) * PS:(2 * i + 2) * PS],
                                         AF.Exp, bias=BIAS[:, 2 + i:3 + i])
                for i in exp_rng:
                    nc.scalar.activation(EX[:, (4 + i) * PS:(5 + i) * PS],
                                         W[:, (2 * i + 1) * PS:(2 * i + 2) * PS],
                                         AF.Exp, bias=BIAS[:, 4 + i:5 + i])

                # ---- pair matrices: planes (i,j), plane q = 2i+j ----
                if "tree" in ablate:
                    continue
                nc.vector.tensor_tensor(
                    out=_ap(PM, 0, [[2 * PS, 2], [1, PS]]),
                    in0=_ap(EX, 0, [[PS, 2], [1, PS]]),
                    in1=_ap(EX, 2 * PS, [[PS, 2], [1, PS]]), op=OP.add)
                # P_i1 = delta*wh_i + wd_i (TS into planes {1,3}, then
                # in-place add of wd)
                nc.vector.tensor_scalar(
                    out=_ap(PM, PS, [[2 * PS, 2], [1, PS]]),
                    in0=_ap(EX, 0, [[PS, 2], [1, PS]]),
                    scalar1=c["delta"], scalar2=None, op0=OP.mult)
                nc.vector.tensor_tensor(
                    out=_ap(PM, PS, [[2 * PS, 2], [1, PS]]),
                    in0=_ap(PM, PS, [[2 * PS, 2], [1, PS]]),
                    in1=_ap(EX, 4 * PS, [[PS, 2], [1, PS]]), op=OP.add)

                # ---- pair-0 patch: alpha-init leaf (j-independent) ----
                T1 = wk.tile([P, 2], BF16, tag="T1")
                nc.vector.tensor_scalar(out=T1,
                                        in0=_ap(EX, 0, [[PS, 2], [1, 1]]),
                                        scalar1=c["p1"], scalar2=None,
                                        op0=OP.mult)
                T2 = wk.tile([P, 2], BF16, tag="T2")
                nc.gpsimd.tensor_tensor(out=T2, in0=T1,
                                        in1=_ap(EX, 2 * PS, [[PS, 2], [1, 1]]),
                                        op=OP.add)
                nc.vector.tensor_scalar(
                    out=_ap(PM, 0, [[2 * PS, 2], [PS, 2]]),
                    in0=_ap(T2, 0, [[1, 2], [0, 2]]),
                    scalar1=c["p2"], scalar2=None, op0=OP.mult)

                # ---- linear tree: 6 levels of 2x2 products ----
                # global bit-reversed storage: every level combines the two
                # contiguous halves of each plane (A = second half = odd
                # children = left factor).
                cur, pl = PM, PS
                for v in range(1, 7):
                    h = pl // 2
                    eng = nc.vector if v <= 3 else nc.gpsimd
                    M1 = wk.tile([P, 4 * h], BF16, tag=f"M1_{v}")
                    M2 = wk.tile([P, 4 * h], BF16, tag=f"M2_{v}")
                    NX = wk.tile([P, 4 * h], BF16, tag=f"NX_{v}")
                    oap = [[2 * h, 2], [h, 2], [1, h]]
                    eng.tensor_tensor(
                        out=_ap(M1, 0, oap),
                        in0=_ap(cur, 0 * pl + h, [[2 * pl, 2], [0, 2], [1, h]]),
                        in1=_ap(cur, 0 * pl + 0, [[0, 2], [pl, 2], [1, h]]),
                        op=OP.mult)
                    eng.tensor_tensor(
                        out=_ap(M2, 0, oap),
                        in0=_ap(cur, 1 * pl + h, [[2 * pl, 2], [0, 2], [1, h]]),
                        in1=_ap(cur, 2 * pl + 0, [[0, 2], [pl, 2], [1, h]]),
                        op=OP.mult)
                    eng.tensor_tensor(out=NX, in0=M1, in1=M2, op=OP.add)
                    cur, pl = NX, h

                # ---- Ln -> f32 span logs into LBA[(i,j)][s*G + g] ----
                nc.scalar.activation(
                    _ap(LBA, g, [[G * NBLK, 4], [G, NBLK]]),
                    _ap(cur, 0, [[NBLK, 4], [1, NBLK]]), AF.Ln)

            # ---- top log tree, batched across groups (g interleaved) ----
            if "tree" in ablate or "top" in ablate:
                nc.sync.dma_start(
                    out=bass.AP(tensor=outp[:].tensor, offset=0,
                                ap=[[4, P], [4 * P, G], [1, 4]]),
                    in_=_ap(ACA, 0, [[4, G], [1, 4]]))
                continue
            tpl_s = NBLK
            n_s = NBLK
            src = LBA
            while n_s > 1:
                h = n_s // 2
                S0 = wk.tile([P, 4 * G * h], F32, tag=f"S0_{n_s}")
                S1 = wk.tile([P, 4 * G * h], F32, tag=f"S1_{n_s}")
                MN = wk.tile([P, 4 * G * h], F32, tag=f"MN_{n_s}")
                oap = [[2 * G * h, 2], [G * h, 2], [1, G * h]]

                def a_ap(mu):
                    return _ap(src, mu * G * tpl_s + G * h,
                               [[2 * G * tpl_s, 2], [0, 2], [1, G * h]])

                def b_ap(mu):
                    return _ap(src, 2 * mu * G * tpl_s + 0,
                               [[0, 2], [G * tpl_s, 2], [1, G * h]])

                nc.gpsimd.tensor_tensor(out=_ap(S0, 0, oap), in0=a_ap(0),
                                        in1=b_ap(0), op=OP.add)
                nc.gpsimd.tensor_tensor(out=_ap(S1, 0, oap), in0=a_ap(1),
                                        in1=b_ap(1), op=OP.add)
                nc.vector.tensor_tensor(out=MN, in0=S0, in1=S1, op=OP.min)
                nc.vector.tensor_tensor(out=S0, in0=S0, in1=S1, op=OP.max)
                nc.gpsimd.tensor_tensor(out=MN, in0=MN, in1=S0, op=OP.subtract)
                nc.scalar.activation(MN, MN, AF.Exp)
                nc.scalar.activation(MN, MN, AF.Ln, bias=1.0)
                nc.gpsimd.tensor_tensor(out=S1, in0=S0, in1=MN, op=OP.add)
                src, tpl_s, n_s = S1, h, h

            # final: lT_i0 for each g -> ACA[:, 4g + i]
            nc.vector.tensor_scalar(
                out=_ap(ACA, 0, [[4, G], [1, 2]]),
                in0=_ap(src, 0, [[1, G], [2 * G, 2]]),
                scalar1=1.0, scalar2=None, op0=OP.mult)
            nc.sync.dma_start(
                out=bass.AP(tensor=outp[:].tensor, offset=0,
                            ap=[[4, P], [4 * P, G], [1, 4]]),
                in_=_ap(ACA, 0, [[4, G], [1, 4]]))

    return _split_multiwaits(nc)


_CACHE = {}
_IDX = None


def _indices():
    global _IDX
    if _IDX is None:
        nb = 11                      # log2(PS): global bit-reversal of pairs
        ks = np.zeros(PS, np.int64)
        for i in range(PS):
            b = 0
            for k in range(nb):
                if i >> k & 1:
                    b |= 1 << (nb - 1 - k)
            ks[i] = b
        _IDX = (2 * ks + 1, 2 * ks)     # hi, lo position indices [2048]
    return _IDX


def _get_nc(key, consts, repeat=1):
    if key not in _CACHE:
        _CACHE[key] = _build(consts, repeat=repeat)
    return _CACHE[key]


def _np_crf_fallback(emissions, tags, mask, transitions, start_transitions,
                     end_transitions):
    em = np.asarray(emissions, np.float64)
    tgn = np.asarray(tags, np.int64)
    mk = np.asarray(mask, bool)
    tr = np.asarray(transitions, np.float64)
    st = np.asarray(start_transitions, np.float64)
    en = np.asarray(end_transitions, np.float64)
    B, Ln, T = em.shape
    score = st[tgn[:, 0]] + em[np.arange(B), 0, tgn[:, 0]]
    for l in range(1, Ln):
        emit = em[np.arange(B), l, tgn[:, l]]
        trans = tr[tgn[:, l], tgn[:, l - 1]]
        score += (emit + trans) * mk[:, l]
    alpha = st[None, :] + em[:, 0]
    for l in range(1, Ln):
        sc = alpha[:, None, :] + tr[None, :, :]
        m = sc.max(axis=2, keepdims=True)
        a_new = np.log(np.exp(sc - m).sum(axis=2)) + m[:, :, 0] + em[:, l]
        alpha = np.where(mk[:, l, None], a_new, alpha)
    m = (alpha + en).max(axis=1, keepdims=True)
    logz = np.log(np.exp(alpha + en - m).sum(axis=1)) + m[:, 0]
    sl = np.maximum(mk.sum(axis=1), 1.0)
    return np.float32(((logz - score) / sl).mean())


def kernel(emissions, tags, mask, transitions, start_transitions,
           end_transitions):
    B, Ln, T = emissions.shape
    if not (T == 2 and Ln == L and B == N_CORES * G * P and np.all(mask)):
        return _np_crf_fallback(emissions, tags, mask, transitions,
                                start_transitions, end_transitions)

    tr = np.asarray(transitions, np.float64)
    st = np.asarray(start_transitions, np.float64)
    en = np.asarray(end_transitions, np.float64)
    CB = 0.9 + float(tr.mean())
    consts = _host_consts(tr, CB)
    nc = _get_nc(consts, consts)

    idx_hi, idx_lo = _indices()
    emf = np.asarray(emissions, np.float32)
    tgn = np.asarray(tags, np.int64)

    # host tag-only gold part (bilinear form of transition sum) + en[t_last]
    a = tgn[:, 1:]
    b = tgn[:, :-1]
    sa = a.sum(1, dtype=np.int64)
    sb = b.sum(1, dtype=np.int64)
    sab = (a * b).sum(1, dtype=np.int64)
    cC = tr[1, 1] - tr[1, 0] - tr[0, 1] + tr[0, 0]
    gtag = (tr[0, 0] * (Ln - 1) + (tr[1, 0] - tr[0, 0]) * sa
            + (tr[0, 1] - tr[0, 0]) * sb + cC * sab + en[tgn[:, -1]])

    BS = G * P
    in_maps = []
    for cidx in range(N_CORES):
        esl = np.array(emf[cidx * BS:(cidx + 1) * BS])     # [BS, L, 2]
        esl[:, 0, :] += st[None, :].astype(np.float32)
        tsl = tgn[cidx * BS:(cidx + 1) * BS]
        EMp = np.empty((BS, 4, PS), NPBF)
        EMp[:, 0] = esl[:, idx_hi, 0]
        EMp[:, 1] = esl[:, idx_hi, 1]
        EMp[:, 2] = esl[:, idx_lo, 0]
        EMp[:, 3] = esl[:, idx_lo, 1]
        TGp = np.empty((BS, 2, PS), NPBF)
        TGp[:, 0] = tsl[:, idx_hi]
        TGp[:, 1] = tsl[:, idx_lo]
        in_maps.append({
            "emissions": np.ascontiguousarray(EMp.reshape(BS, 4 * PS)),
            "tagsf": np.ascontiguousarray(TGp.reshape(BS, 2 * PS)),
        })

    res = run_bass_kernel_spmd(nc, in_maps, core_ids=list(range(N_CORES)))
    outs = np.concatenate([r["outp"] for r in res.results])   # [B, 4]
    lt0 = outs[:, 0].astype(np.float64)
    lt1 = outs[:, 1].astype(np.float64)
    se0 = outs[:, 2].astype(np.float64)
    stD = outs[:, 3].astype(np.float64)
    a0 = en[0] + lt0
    a1 = en[1] + lt1
    mx = np.maximum(a0, a1)
    logZ = CB * Ln + mx + np.log1p(np.exp(np.minimum(a0, a1) - mx))
    gold = se0 + stD + gtag
    nll = (logZ - gold) / Ln
    return np.float32(nll.mean())
